# revision 10
# baseline (speedup 1.0000x reference)
"""Trainium2 Bass kernel for BaseGenerator: mapped = mapping @ base_flat.

Strategy (8-core SPMD, pure data-parallel over output pixels):
  - mapping [P1=16384, P0=16384] f32 is row-sharded: core c owns output rows
    [c*2048, (c+1)*2048).  Host pre-transposes each shard to mt_c [P0, 2048]
    (K-major) so the device streams it with contiguous 1 MiB DMAs and the
    contraction axis lands on SBUF partitions.
  - base_flat [P0, 3] is replicated, rearranged host-side to [128, 128*3] so
    each 128-row K-chunk is a [128, 3] stationary matmul operand (lhsT).
  - Device: for each of 128 K-chunks, DMA mt tile [128, 2048], then 4 matmuls
    (N=512) accumulate into 4 PSUM banks of [3, 512] f32.  Epilogue copies
    PSUM -> SBUF -> DRAM out [3, 2048] f32.
  - Host concatenates per-core outputs -> [16384, 3] -> [128, 128, 3].

The kernel is DMA-bound: 128 MiB/core f32 (~360 GB/s/core HBM) or 64 MiB/core
in bf16.  COMPUTE_DTYPE selects the internal precision of the mapping/base
operands; accumulation is always f32 in PSUM and the output is always f32.
"""

import sys

import numpy as np

try:
    import concourse.bacc as bacc
except ImportError:  # fresh env without PYTHONPATH: fall back to repo paths
    for _p in ("/opt/trn_rl_repo", "/opt/pypackages",
               "/root/.axon_site/_ro/trn_rl_repo",
               "/root/.axon_site/_ro/pypackages"):
        if _p not in sys.path:
            sys.path.append(_p)
    import concourse.bacc as bacc
import concourse.bass as bass
import concourse.mybir as mybir
import concourse.tile as tile
from concourse.bass_utils import run_bass_kernel_spmd

H0 = W0 = 128
H1 = W1 = 128
P0 = H0 * W0          # 16384 contraction length
P1 = H1 * W1          # 16384 output pixels
N_CORES = 8
N_PER_CORE = P1 // N_CORES   # 2048 output pixels per core
KC = 128              # K-chunk size (SBUF partitions)
N_KCHUNKS = P0 // KC  # 128
NB = 512              # matmul moving free dim (one PSUM bank of f32)
N_BANKS = N_PER_CORE // NB   # 4

COMPUTE_DTYPE = "bfloat16"   # "float32" or "bfloat16"
CHUNKS_PER_DMA = 8           # K-chunks fetched per dma_start (8 -> 4 MiB bf16)
DMA_BUFS = 4                 # in-flight DMA tiles
ALT_DMA_RINGS = False         # alternate sync/scalar HWDGE rings

_PROGRAM_CACHE = {}


def _np_compute_dtype(name):
    if name == "float32":
        return np.float32
    import ml_dtypes
    return ml_dtypes.bfloat16


def _build_program(dtype_name):
    """Build + compile the SPMD Bass program (identical on all 8 cores)."""
    dt = getattr(mybir.dt, dtype_name)
    nc = bacc.Bacc(
        "TRN2", target_bir_lowering=False, debug=False, num_devices=N_CORES
    )
    mt = nc.dram_tensor("mt", [P0, N_PER_CORE], dt, kind="ExternalInput")
    bt = nc.dram_tensor("bt", [KC, N_KCHUNKS * 3], dt, kind="ExternalInput")
    out = nc.dram_tensor(
        "out", [3, N_PER_CORE], mybir.dt.float32, kind="ExternalOutput"
    )

    qc = CHUNKS_PER_DMA
    n_dmas = N_KCHUNKS // qc
    # mt[(a*KC)+p, n] viewed as [p, (a n)] so a slice of the free dim covers
    # `qc` consecutive K-chunks in one contiguous-per-partition DMA.
    with tile.TileContext(nc) as tc:
        with (
            tc.tile_pool(name="bpool", bufs=1) as bpool,
            tc.tile_pool(name="mpool", bufs=DMA_BUFS) as mpool,
            tc.tile_pool(name="psum", bufs=1, space=bass.MemorySpace.PSUM) as pp,
            tc.tile_pool(name="opool", bufs=1) as opool,
        ):
            b_sb = bpool.tile([KC, N_KCHUNKS * 3], dt)
            nc.sync.dma_start(b_sb[:], bt[:])

            ps = [
                pp.tile([3, NB], mybir.dt.float32, name=f"ps{i}", tag=f"ps{i}")
                for i in range(N_BANKS)
            ]

            mt_v = mt.rearrange("(a p) n -> a p n", p=KC)  # [128, 128, 2048]

            def do_chunks(m_sb, k_first, n_chunks):
                for a in range(n_chunks):
                    k1 = k_first + a
                    lhsT = b_sb[:, k1 * 3:(k1 + 1) * 3]
                    for nb in range(N_BANKS):
                        nc.tensor.matmul(
                            ps[nb][:, :],
                            lhsT,
                            m_sb[:, a * N_PER_CORE + nb * NB:
                                 a * N_PER_CORE + (nb + 1) * NB],
                            start=(k1 == 0),
                            stop=(k1 == N_KCHUNKS - 1),
                        )

            # Prologue: first `qc` chunks arrive as small single-chunk DMAs so
            # the PE starts as soon as the first 512 KB lands instead of
            # waiting for a full qc-chunk tile.
            for k1 in range(qc):
                m_pro = mpool.tile(
                    [KC, N_PER_CORE], dt, name="m_pro", tag="m_pro", bufs=2
                )
                nc.sync.dma_start(m_pro[:], mt_v[k1])
                do_chunks(m_pro, k1, 1)

            for i in range(1, n_dmas):
                m_sb = mpool.tile([KC, qc * N_PER_CORE], dt, name="m_sb")
                nc.sync.dma_start(
                    m_sb.rearrange("p (a n) -> p a n", a=qc),
                    mt_v[i * qc:(i + 1) * qc].rearrange("a p n -> p a n"),
                )
                do_chunks(m_sb, i * qc, qc)

            o_sb = opool.tile([3, N_PER_CORE], mybir.dt.float32)
            for nb in range(N_BANKS):
                nc.vector.tensor_copy(o_sb[:, nb * NB:(nb + 1) * NB], ps[nb][:, :])
            nc.sync.dma_start(out[:], o_sb[:])

    nc.compile()
    return nc


def _get_program(dtype_name):
    if dtype_name not in _PROGRAM_CACHE:
        _PROGRAM_CACHE[dtype_name] = _build_program(dtype_name)
    return _PROGRAM_CACHE[dtype_name]


def _prepare_inputs(mapping, base_image, dtype_name):
    np_dt = _np_compute_dtype(dtype_name)
    # base [128,128,3] -> base_flat [P0, 3] -> [128 part, 128 kchunk * 3]
    # bt[p, k1*3 + c] = base_flat[k1*128 + p, c]
    base_flat = np.asarray(base_image, dtype=np.float32).reshape(P0, 3)
    bt = np.ascontiguousarray(
        base_flat.reshape(N_KCHUNKS, KC, 3).transpose(1, 0, 2).reshape(
            KC, N_KCHUNKS * 3
        )
    ).astype(np_dt)

    in_maps = []
    for c in range(N_CORES):
        shard = mapping[c * N_PER_CORE:(c + 1) * N_PER_CORE, :]  # [2048, P0] view
        mt_c = shard.T.astype(np_dt)  # one pass: strided read + cast + contig write
        in_maps.append({"mt": mt_c, "bt": bt})
    return in_maps


def _run(mapping, base_image, dtype_name, trace=False):
    nc = _get_program(dtype_name)
    in_maps = _prepare_inputs(mapping, base_image, dtype_name)
    res = run_bass_kernel_spmd(nc, in_maps, list(range(N_CORES)), trace=trace)
    mapped_flat = np.concatenate(
        [res.results[c]["out"].T for c in range(N_CORES)], axis=0
    )  # [P1, 3] f32
    mapped_image = mapped_flat.reshape(H1, W1, 3)
    return mapped_image, res


def kernel(mapping, base_image):
    mapping = np.asarray(mapping, dtype=np.float32)
    base_image = np.asarray(base_image, dtype=np.float32)
    mapped_image, _ = _run(mapping, base_image, COMPUTE_DTYPE)
    return (base_image, mapped_image)


# revision 12
# speedup vs baseline: 1.1129x; 1.1129x over previous
"""Trainium2 Bass kernel for BaseGenerator: mapped = mapping @ base_flat.

Strategy (8-core SPMD, pure data-parallel over output pixels):
  - mapping [P1=16384, P0=16384] f32 is row-sharded: core c owns output rows
    [c*2048, (c+1)*2048).  Host pre-transposes each shard to mt_c [P0, 2048]
    (K-major) so the device streams it with contiguous 1 MiB DMAs and the
    contraction axis lands on SBUF partitions.
  - base_flat [P0, 3] is replicated, rearranged host-side to [128, 128*3] so
    each 128-row K-chunk is a [128, 3] stationary matmul operand (lhsT).
  - Device: for each of 128 K-chunks, DMA mt tile [128, 2048], then 4 matmuls
    (N=512) accumulate into 4 PSUM banks of [3, 512] f32.  Epilogue copies
    PSUM -> SBUF -> DRAM out [3, 2048] f32.
  - Host concatenates per-core outputs -> [16384, 3] -> [128, 128, 3].

The kernel is DMA-bound: 128 MiB/core f32 (~360 GB/s/core HBM) or 64 MiB/core
in bf16.  COMPUTE_DTYPE selects the internal precision of the mapping/base
operands; accumulation is always f32 in PSUM and the output is always f32.
"""

import sys

import numpy as np

try:
    import concourse.bacc as bacc
except ImportError:  # fresh env without PYTHONPATH: fall back to repo paths
    for _p in ("/opt/trn_rl_repo", "/opt/pypackages",
               "/root/.axon_site/_ro/trn_rl_repo",
               "/root/.axon_site/_ro/pypackages"):
        if _p not in sys.path:
            sys.path.append(_p)
    import concourse.bacc as bacc
import concourse.bass as bass
import concourse.mybir as mybir
import concourse.tile as tile
from concourse.bass_utils import run_bass_kernel_spmd
from concourse.tile_rust import add_dep_helper

H0 = W0 = 128
H1 = W1 = 128
P0 = H0 * W0          # 16384 contraction length
P1 = H1 * W1          # 16384 output pixels
N_CORES = 8
N_PER_CORE = P1 // N_CORES   # 2048 output pixels per core
KC = 128              # K-chunk size (SBUF partitions)
N_KCHUNKS = P0 // KC  # 128
NB = 512              # matmul moving free dim (one PSUM bank of f32)
N_BANKS = N_PER_CORE // NB   # 4

COMPUTE_DTYPE = "bfloat16"   # "float32" or "bfloat16"
CHUNKS_PER_DMA = 8           # K-chunks fetched per dma_start (8 -> 4 MiB bf16)
DMA_BUFS = 4                 # in-flight DMA tiles
ALT_DMA_RINGS = False        # alternate sync/scalar HWDGE rings (crashes HW; keep off)
DEDUP_LDWEIGHTS = True       # 1 LDWEIGHTS per K-chunk instead of per matmul

_PROGRAM_CACHE = {}


def _np_compute_dtype(name):
    if name == "float32":
        return np.float32
    import ml_dtypes
    return ml_dtypes.bfloat16


def _build_program(dtype_name):
    """Build + compile the SPMD Bass program (identical on all 8 cores)."""
    dt = getattr(mybir.dt, dtype_name)
    nc = bacc.Bacc(
        "TRN2", target_bir_lowering=False, debug=False, num_devices=N_CORES
    )
    mt = nc.dram_tensor("mt", [P0, N_PER_CORE], dt, kind="ExternalInput")
    bt = nc.dram_tensor("bt", [KC, N_KCHUNKS * 3], dt, kind="ExternalInput")
    out = nc.dram_tensor(
        "out", [3, N_PER_CORE], mybir.dt.float32, kind="ExternalOutput"
    )

    qc = CHUNKS_PER_DMA
    n_dmas = N_KCHUNKS // qc
    # mt[(a*KC)+p, n] viewed as [p, (a n)] so a slice of the free dim covers
    # `qc` consecutive K-chunks in one contiguous-per-partition DMA.
    with tile.TileContext(nc) as tc:
        with (
            tc.tile_pool(name="bpool", bufs=1) as bpool,
            tc.tile_pool(name="mpool", bufs=DMA_BUFS) as mpool,
            tc.tile_pool(name="psum", bufs=1, space=bass.MemorySpace.PSUM) as pp,
            tc.tile_pool(name="opool", bufs=1) as opool,
        ):
            b_sb = bpool.tile([KC, N_KCHUNKS * 3], dt)
            nc.sync.dma_start(b_sb[:], bt[:])

            ps = [
                pp.tile([3, NB], mybir.dt.float32, name=f"ps{i}", tag=f"ps{i}")
                for i in range(N_BANKS)
            ]

            mt_v = mt.rearrange("(a p) n -> a p n", p=KC)  # [128, 128, 2048]
            prev_mm = None
            for i in range(n_dmas):
                m_sb = mpool.tile([KC, qc * N_PER_CORE], dt, name="m_sb")
                nc.sync.dma_start(
                    m_sb.rearrange("p (a n) -> p a n", a=qc),
                    mt_v[i * qc:(i + 1) * qc].rearrange("a p n -> p a n"),
                )
                for a in range(qc):
                    k1 = i * qc + a
                    lhsT = b_sb[:, k1 * 3:(k1 + 1) * 3]
                    for nb in range(N_BANKS):
                        mm = nc.tensor.matmul(
                            ps[nb][:, :],
                            lhsT,
                            m_sb[:, a * N_PER_CORE + nb * NB:
                                 a * N_PER_CORE + (nb + 1) * NB],
                            start=(k1 == 0),
                            stop=(k1 == N_KCHUNKS - 1),
                        )
                        if DEDUP_LDWEIGHTS:
                            # matmuls 2-4 of a K-chunk reuse the stationary
                            # operand loaded by the first; chain PE program
                            # order explicitly so the reuse stays valid.
                            if nb > 0:
                                mm.ins.ldweights = False
                            if prev_mm is not None:
                                add_dep_helper(
                                    mm.ins, prev_mm.ins, sync=False,
                                    reason="pe-order for weight reuse",
                                )
                            prev_mm = mm

            o_sb = opool.tile([3, N_PER_CORE], mybir.dt.float32)
            for nb in range(N_BANKS):
                nc.vector.tensor_copy(o_sb[:, nb * NB:(nb + 1) * NB], ps[nb][:, :])
            nc.sync.dma_start(out[:], o_sb[:])

    nc.compile()
    return nc


def _get_program(dtype_name):
    if dtype_name not in _PROGRAM_CACHE:
        _PROGRAM_CACHE[dtype_name] = _build_program(dtype_name)
    return _PROGRAM_CACHE[dtype_name]


def _prepare_inputs(mapping, base_image, dtype_name):
    np_dt = _np_compute_dtype(dtype_name)
    # base [128,128,3] -> base_flat [P0, 3] -> [128 part, 128 kchunk * 3]
    # bt[p, k1*3 + c] = base_flat[k1*128 + p, c]
    base_flat = np.asarray(base_image, dtype=np.float32).reshape(P0, 3)
    bt = np.ascontiguousarray(
        base_flat.reshape(N_KCHUNKS, KC, 3).transpose(1, 0, 2).reshape(
            KC, N_KCHUNKS * 3
        )
    ).astype(np_dt)

    in_maps = []
    for c in range(N_CORES):
        shard = mapping[c * N_PER_CORE:(c + 1) * N_PER_CORE, :]  # [2048, P0] view
        mt_c = shard.T.astype(np_dt)  # one pass: strided read + cast + contig write
        in_maps.append({"mt": mt_c, "bt": bt})
    return in_maps


def _run(mapping, base_image, dtype_name, trace=False):
    nc = _get_program(dtype_name)
    in_maps = _prepare_inputs(mapping, base_image, dtype_name)
    res = run_bass_kernel_spmd(nc, in_maps, list(range(N_CORES)), trace=trace)
    mapped_flat = np.concatenate(
        [res.results[c]["out"].T for c in range(N_CORES)], axis=0
    )  # [P1, 3] f32
    mapped_image = mapped_flat.reshape(H1, W1, 3)
    return mapped_image, res


def kernel(mapping, base_image):
    mapping = np.asarray(mapping, dtype=np.float32)
    base_image = np.asarray(base_image, dtype=np.float32)
    mapped_image, _ = _run(mapping, base_image, COMPUTE_DTYPE)
    return (base_image, mapped_image)
